# revision 15
# baseline (speedup 1.0000x reference)
"""GQA attention kernel for 8 trn2 NeuronCores.

Sharding: tensor-parallel over the 8 KV groups (1 group = 4 Q heads per
core, both batch elements), then an AllToAll reshards the per-core
context [256 feat, 4096 rows] into row-shards [2048 feat, 512 rows] so
the output projection runs row-parallel with no reduction.

Shapes (hardcoded): B=2, S=2048, D=2048, H=32, G=8, HD=64.
"""

import numpy as np
import concourse.bass as bass
import concourse.mybir as mybir
import concourse.tile as tile
from concourse import bacc
from concourse.bass import broadcast_tensor_aps
from concourse.bass_utils import run_bass_kernel_spmd
from concourse.masks import make_identity

N_CORES = 8
B, S, D = 2, 2048, 2048
H, G, HD = 32, 8, 64
GS = H // G                       # 4 q heads per kv group
ROWS = B * S                      # 4096 flattened (b, s) rows
RPC = ROWS // N_CORES             # 512 output rows per core
EPS = 1e-6
F32 = mybir.dt.float32
AX = mybir.AxisListType
ALU = mybir.AluOpType
AF = mybir.ActivationFunctionType

KB = D // 128                     # 16 contraction blocks for projections
MB = ROWS // 128                  # 32 row blocks
QKV = GS * HD + 2 * HD            # 384 projected features per core
NQK = GS + 1                      # 5 heads that get rmsnorm+rope (4 q + 1 k)
SQT = 512                         # attention query-tile width
SKT = 128                         # attention key-tile height
JQ = S // SQT                     # 4 query tiles per batch
IKB = S // SKT                    # 16 key blocks per batch


def _build():
    nc = bacc.Bacc(num_devices=N_CORES)

    xT = nc.dram_tensor("xT", [D, ROWS], F32, kind="ExternalInput")
    wqkv = nc.dram_tensor("wqkv", [D, QKV], F32, kind="ExternalInput")
    wo = nc.dram_tensor("wo", [H * HD, D], F32, kind="ExternalInput")
    cs = nc.dram_tensor("cs", [S, HD], F32, kind="ExternalInput")
    sn = nc.dram_tensor("sn", [S, HD], F32, kind="ExternalInput")
    wvec = nc.dram_tensor("wvec", [1, NQK * HD], F32, kind="ExternalInput")
    maskM = nc.dram_tensor("maskM", [128, 1024], F32, kind="ExternalInput")
    out_rows = nc.dram_tensor("out_rows", [RPC, D], F32, kind="ExternalOutput")

    with tile.TileContext(nc) as tc:
        with (
            tc.tile_pool(name="const", bufs=1) as const,
            tc.tile_pool(name="dram", bufs=1, space="DRAM") as dram,
        ):
            a2a_in = dram.tile([N_CORES, GS * HD, RPC], F32)
            a2a_out = dram.tile([N_CORES, GS * HD, RPC], F32)

            w_sb = const.tile([128, KB, QKV], F32)
            nc.sync.dma_start(w_sb[:], wqkv[:].rearrange("(k p) j -> p k j", p=128))
            cos_sb = const.tile([128, S // 128, HD], F32)
            sin_sb = const.tile([128, S // 128, HD], F32)
            nc.sync.dma_start(cos_sb[:], cs[:].rearrange("(m p) d -> p m d", p=128))
            nc.sync.dma_start(sin_sb[:], sn[:].rearrange("(m p) d -> p m d", p=128))
            mask_sb = const.tile([128, 1024], F32)
            nc.sync.dma_start(mask_sb[:], maskM[:])
            ident = const.tile([128, 128], F32)
            make_identity(nc, ident)
            wv1 = const.tile([1, NQK * HD], F32)
            nc.sync.dma_start(wv1[:], wvec[:])
            wv_sb = const.tile([128, NQK * HD], F32)
            nc.gpsimd.partition_broadcast(wv_sb[:], wv1[0:1, :])
            eps_sb = const.tile([128, 1], F32)
            nc.vector.memset(eps_sb[:], EPS)

            # persistent activations
            qT_a = const.tile([128, ROWS], F32)   # heads 0,1 stacked (qT)
            qT_b = const.tile([128, ROWS], F32)   # heads 2,3 stacked
            kT_sb = const.tile([128, ROWS], F32)  # kT duplicated in both halves
            v1_sb = const.tile([128, MB, HD + 1], F32)
            nc.vector.memset(v1_sb[:], 1.0)       # col 64 stays 1.0 (denominator trick)

            # ---------------- phase 1: qkv proj + rmsnorm + rope + transpose
            with (
                tc.tile_pool(name="xs", bufs=20) as xs,
                tc.tile_pool(name="pp", bufs=3, space="PSUM") as ppool,
                tc.tile_pool(name="tp", bufs=2, space="PSUM") as tpool,
                tc.tile_pool(name="ev", bufs=3) as ev,
            ):
                for m4 in range(MB // 4):
                    xts = []
                    for k in range(KB):
                        t = xs.tile([128, 512], F32, tag="xts")
                        nc.sync.dma_start(
                            t[:], xT[k * 128:(k + 1) * 128, m4 * 512:(m4 + 1) * 512]
                        )
                        xts.append(t)
                    for i in range(4):
                        m = m4 * 4 + i
                        pp = ppool.tile([128, QKV], F32, tag="pp")
                        for k in range(KB):
                            nc.tensor.matmul(
                                pp[:],
                                xts[k][:, i * 128:(i + 1) * 128],
                                w_sb[:, k, :],
                                start=(k == 0),
                                stop=(k == KB - 1),
                            )
                        # rmsnorm over each 64-wide head slice (q0..q3, k)
                        nqk = NQK * HD
                        sq = ev.tile([128, nqk], F32, tag="sq")
                        nc.scalar.activation(sq[:], pp[:, :nqk], AF.Square)
                        ssum = ev.tile([128, NQK], F32, tag="ssum")
                        nc.vector.tensor_reduce(
                            ssum[:], sq[:].rearrange("p (h d) -> p h d", d=HD),
                            AX.X, ALU.add,
                        )
                        srt = ev.tile([128, NQK], F32, tag="srt")
                        nc.scalar.activation(srt[:], ssum[:], AF.Sqrt,
                                             bias=eps_sb[:], scale=1.0 / HD)
                        rs = ev.tile([128, NQK], F32, tag="rs")
                        nc.vector.reciprocal(rs[:], srt[:])
                        qkn = ev.tile([128, nqk], F32, tag="qkn")
                        for h in range(NQK):
                            nc.vector.tensor_scalar_mul(
                                qkn[:, h * HD:(h + 1) * HD],
                                pp[:, h * HD:(h + 1) * HD],
                                rs[:, h:h + 1],
                            )
                        nc.vector.tensor_mul(qkn[:], qkn[:], wv_sb[:])
                        # rope (rotate-half) on all 5 heads at once
                        sm = m % (S // 128)
                        hf = HD // 2
                        qv = qkn[:].rearrange("p (h d) -> p h d", d=HD)
                        qkr = ev.tile([128, nqk], F32, tag="qkr")
                        rv = qkr[:].rearrange("p (h d) -> p h d", d=HD)
                        t1 = ev.tile([128, NQK, hf], F32, tag="t1")
                        t2 = ev.tile([128, NQK, hf], F32, tag="t2")

                        def bmul(out_ap, a_ap, trig, lo):
                            tr = trig[:, sm:sm + 1, lo * hf:(lo + 1) * hf]
                            a2, b2 = broadcast_tensor_aps(a_ap, tr)
                            nc.vector.tensor_tensor(out_ap, a2, b2, ALU.mult)

                        lo_in = qv[:, :, 0:hf]
                        hi_in = qv[:, :, hf:HD]
                        bmul(t1[:], hi_in, sin_sb, 0)        # x2 * sin_lo
                        bmul(t2[:], lo_in, sin_sb, 1)        # x1 * sin_hi
                        bmul(rv[:, :, 0:hf], lo_in, cos_sb, 0)
                        bmul(rv[:, :, hf:HD], hi_in, cos_sb, 1)
                        nc.vector.tensor_sub(rv[:, :, 0:hf], rv[:, :, 0:hf], t1[:])
                        nc.vector.tensor_add(rv[:, :, hf:HD], rv[:, :, hf:HD], t2[:])
                        # v straight from psum (no norm/rope)
                        nc.scalar.activation(v1_sb[:, m, 0:HD], pp[:, nqk:QKV], AF.Copy)
                        # transposes: [seq,hd] -> [hd,seq]
                        tq1 = tpool.tile([128, 128], F32, tag="tq")
                        nc.tensor.transpose(tq1[:], qkr[:, 0:128], ident[:])
                        nc.scalar.activation(qT_a[:, m * 128:(m + 1) * 128], tq1[:], AF.Copy)
                        tq2 = tpool.tile([128, 128], F32, tag="tq")
                        nc.tensor.transpose(tq2[:], qkr[:, 128:256], ident[:])
                        nc.scalar.activation(qT_b[:, m * 128:(m + 1) * 128], tq2[:], AF.Copy)
                        kst = ev.tile([128, 128], F32, tag="kst")
                        nc.vector.tensor_copy(kst[:, 0:64], qkr[:, 256:320])
                        nc.vector.tensor_copy(kst[:, 64:128], qkr[:, 256:320])
                        tq3 = tpool.tile([128, 128], F32, tag="tq")
                        nc.tensor.transpose(tq3[:], kst[:], ident[:])
                        nc.scalar.activation(kT_sb[:, m * 128:(m + 1) * 128], tq3[:], AF.Copy)

            # ---------------- phase 2: attention
            with (
                tc.tile_pool(name="ps", bufs=3, space="PSUM") as pspool,
                tc.tile_pool(name="pc", bufs=2, space="PSUM") as pcpool,
                tc.tile_pool(name="ex", bufs=4) as ex,
                tc.tile_pool(name="cn", bufs=3) as cn,
            ):
                for b in range(B):
                    for h in range(GS):
                        qT_t = qT_a if h < 2 else qT_b
                        hp = (h % 2) * 64
                        for jq in range(JQ):
                            q_rhs = qT_t[hp:hp + 64,
                                         b * S + jq * SQT: b * S + (jq + 1) * SQT]
                            pctx = pcpool.tile([HD + 1, SQT], F32, tag="pctx")
                            nkb = (jq + 1) * (SQT // SKT)
                            for ik in range(nkb):
                                pss = pspool.tile([128, SQT], F32, tag="pss")
                                k_lhs = kT_sb[hp:hp + 64,
                                              b * S + ik * SKT: b * S + (ik + 1) * SKT]
                                nc.tensor.matmul(pss[:], k_lhs, q_rhs,
                                                 start=True, stop=True)
                                es = ex.tile([128, SQT], F32, tag="es")
                                nc.scalar.activation(es[:], pss[:], AF.Exp,
                                                     scale=1.0 / np.sqrt(HD))
                                dd = ik * SKT - jq * SQT
                                if dd >= 0:
                                    off = 512 - dd
                                    nc.vector.tensor_mul(
                                        es[:], es[:], mask_sb[:, off:off + SQT])
                                nc.tensor.matmul(
                                    pctx[:],
                                    v1_sb[:, b * (S // 128) + ik, :],
                                    es[:],
                                    start=(ik == 0),
                                    stop=(ik == nkb - 1),
                                )
                            rr = cn.tile([1, SQT], F32, tag="rr")
                            nc.vector.reciprocal(rr[:], pctx[HD:HD + 1, :])
                            rbc = cn.tile([64, SQT], F32, tag="rbc")
                            nc.gpsimd.partition_broadcast(rbc[:], rr[0:1, :])
                            ctxn = cn.tile([64, SQT], F32, tag="ctxn")
                            nc.vector.tensor_mul(ctxn[:], pctx[0:HD, :], rbc[:])
                            nc.sync.dma_start(
                                a2a_in[b * JQ + jq, h * HD:(h + 1) * HD, :], ctxn[:])

            # ---------------- phase 3: all-to-all reshard + out projection
            nc.gpsimd.collective_compute(
                "AllToAll",
                ALU.bypass,
                replica_groups=[list(range(N_CORES))],
                ins=[a2a_in.opt()],
                outs=[a2a_out.opt()],
            )
            ctx_flat = a2a_out[:].rearrange("g f r -> (g f) r")
            with (
                tc.tile_pool(name="cx", bufs=1) as cx,
                tc.tile_pool(name="ws", bufs=4) as ws,
                tc.tile_pool(name="po", bufs=5, space="PSUM") as popool,
                tc.tile_pool(name="ou", bufs=3) as ou,
            ):
                cxt = []
                for k in range(KB):
                    t = cx.tile([128, RPC], F32, tag=f"cx{k}")
                    nc.sync.dma_start(t[:], ctx_flat[k * 128:(k + 1) * 128, :])
                    cxt.append(t)
                for n in range(D // 512):
                    pos = [popool.tile([128, 512], F32, tag="po", name=f"po{n}_{i}")
                           for i in range(4)]
                    for k in range(KB):
                        wt = ws.tile([128, 512], F32, tag="wt")
                        nc.sync.dma_start(
                            wt[:], wo[k * 128:(k + 1) * 128, n * 512:(n + 1) * 512])
                        for mi in range(4):
                            nc.tensor.matmul(
                                pos[mi][:],
                                cxt[k][:, mi * 128:(mi + 1) * 128],
                                wt[:],
                                start=(k == 0),
                                stop=(k == KB - 1),
                            )
                    for mi in range(4):
                        ot = ou.tile([128, 512], F32, tag="ot")
                        nc.scalar.activation(ot[:], pos[mi][:], AF.Copy)
                        nc.sync.dma_start(
                            out_rows[mi * 128:(mi + 1) * 128, n * 512:(n + 1) * 512],
                            ot[:])

    nc.finalize()
    return nc


_NC_CACHE = None


def _get_nc():
    global _NC_CACHE
    if _NC_CACHE is None:
        _NC_CACHE = _build()
    return _NC_CACHE


def _host_prep(x, cos, sin, Wq, Wk, Wv, Wo, q_norm_w, k_norm_w):
    xT = np.ascontiguousarray(
        np.asarray(x, np.float32).transpose(2, 0, 1).reshape(D, ROWS))
    f = np.arange(1024)[None, :]
    p = np.arange(128)[:, None]
    maskM = (p + 512 <= f).astype(np.float32)
    wvec = np.concatenate(
        [np.tile(np.asarray(q_norm_w, np.float32), GS),
         np.asarray(k_norm_w, np.float32)]).reshape(1, NQK * HD)
    base = dict(
        cs=np.ascontiguousarray(np.asarray(cos, np.float32)),
        sn=np.ascontiguousarray(np.asarray(sin, np.float32)),
        maskM=maskM,
        wvec=np.ascontiguousarray(wvec),
        xT=xT,
    )
    wo_c = np.ascontiguousarray(np.asarray(Wo, np.float32))
    in_maps = []
    for c in range(N_CORES):
        wqkv = np.concatenate(
            [np.asarray(Wq, np.float32)[:, c * GS * HD:(c + 1) * GS * HD],
             np.asarray(Wk, np.float32)[:, c * HD:(c + 1) * HD],
             np.asarray(Wv, np.float32)[:, c * HD:(c + 1) * HD]], axis=1)
        in_maps.append(dict(base, wqkv=np.ascontiguousarray(wqkv), wo=wo_c))
    return in_maps


def kernel(x, mask, cos, sin, Wq, Wk, Wv, Wo, q_norm_w, k_norm_w, _trace=False,
           **kw):
    nc = _get_nc()
    in_maps = _host_prep(x, cos, sin, Wq, Wk, Wv, Wo, q_norm_w, k_norm_w)
    res = run_bass_kernel_spmd(nc, in_maps, list(range(N_CORES)), trace=_trace,
                               **kw)
    out = np.concatenate([res.results[c]["out_rows"] for c in range(N_CORES)],
                         axis=0)
    out = out.reshape(B, S, D).astype(np.float32)
    if _trace:
        return out, res
    return out
